# revision 1
# baseline (speedup 1.0000x reference)
"""DVBF (Deep Variational Bayes Filter) Trainium2 kernel.

Strategy: pure data-parallel over batch (128 -> 16 rows x 8 cores).
Per core, with rows ordered t-major (row r = t*16 + b, 1024 rows):
  Phase A (batched over t): encode all timesteps from data
        h1T = relu(w1.T @ xT + b1)  [512, R] ; z-head -> zT_all [3+1, R]
  Phase B (sequential, 64 steps, transposed layout [feat, 16]):
        LSTM cell (hid=128 = partition dim), beta head, transition MLP
  Phase C (batched): decode h2T = relu(dec_w1.T @ znT), xhat = sigmoid(h2 @ dec_w2 + b2)
  t=63 feedback: one-step transposed decode->sigmoid->encode chain.
All host-side prep (sharding, transposes, bf16 casts, weight layouts) is done
in numpy here; x_gt output is a pure gather of the input computed on host.
"""
import os
import sys
for _p in ('/root/.axon_site', '/root/.axon_site/_ro/trn_rl_repo', '/root/.axon_site/_ro/pypackages'):
    if _p not in sys.path and os.path.isdir(_p):
        sys.path.append(_p)
import numpy as np
import ml_dtypes

import concourse.bass as bass
import concourse.bacc as bacc
import concourse.mybir as mybir
import concourse.tile as tile
from concourse.tile_rust import add_dep_helper
from concourse.bass_utils import run_bass_kernel_spmd

F32 = mybir.dt.float32
F32R = mybir.dt.float32r
BF16 = mybir.dt.bfloat16
AF = mybir.ActivationFunctionType
ALU = mybir.AluOpType

# problem constants (hardcoded per spec)
B_FULL, T, XD, ENC_H, HID, DZ, DB = 128, 64, 4096, 512, 128, 3, 3
N_CORES = 8
BS = B_FULL // N_CORES          # 16 batch rows per core
R = BS * T                      # 1024 rows per core (t-major)
DT_STEP = 0.05
H_STEPS = 64
KX = XD // 128                  # 32 k-chunks of encoder contraction
MH = ENC_H // 128               # 4 m-chunks of hidden 512
NG = 2                          # phase A row groups (N=512 each)
GN = R // NG                    # 512 rows per group
NXD = XD // 512                 # 8 xd slices of 512 for decode out


def _r2(x):
    return np.ascontiguousarray(x)


def build_nc(repeats: int = 1, db2_bias: bool = True, skip_b: bool = False, skip_ac: bool = False) -> bass.Bass:
    nc = bacc.Bacc(None, target_bir_lowering=False, debug=False)

    # ---- DRAM I/O ----
    xTb = nc.declare_dram_parameter("xTb", [XD, R], BF16, isOutput=False)
    ezT = nc.declare_dram_parameter("ezT", [DZ, R], F32, isOutput=False)
    ebT = nc.declare_dram_parameter("ebT", [DB, R], F32, isOutput=False)
    uTf = nc.declare_dram_parameter("uTf", [1, R], BF16, isOutput=False)
    w1T = nc.declare_dram_parameter("w1T", [XD, ENC_H], BF16, isOutput=False)
    eb1r = nc.declare_dram_parameter("eb1r", [1, ENC_H], F32, isOutput=False)   # enc_b1 row
    ew2mu = nc.declare_dram_parameter("ew2mu", [ENC_H, DZ], BF16, isOutput=False)
    ew2ls = nc.declare_dram_parameter("ew2ls", [ENC_H, DZ], BF16, isOutput=False)
    eb2mu = nc.declare_dram_parameter("eb2mu", [DZ, 1], F32, isOutput=False)
    eb2ls = nc.declare_dram_parameter("eb2ls", [DZ, 1], F32, isOutput=False)
    whhT = nc.declare_dram_parameter("whhT", [HID, 4 * HID], BF16, isOutput=False)  # cols [i|f|o|g]
    wihT = nc.declare_dram_parameter("wihT", [DZ + 1, 4 * HID], BF16, isOutput=False)  # rows [z;bias]
    fcr2 = nc.declare_dram_parameter("fcr2", [HID, 35], BF16, isOutput=False)
    fcbmu = nc.declare_dram_parameter("fcbmu", [DB, 1], F32, isOutput=False)
    fcbls = nc.declare_dram_parameter("fcbls", [DB, 1], F32, isOutput=False)
    twz5 = nc.declare_dram_parameter("twz5", [DZ + 2, 32], BF16, isOutput=False)  # rows [wz3; t1_b; wu]
    twb = nc.declare_dram_parameter("twb", [DB, 32], BF16, isOutput=False)
    onesd = nc.declare_dram_parameter("onesd", [1, GN], BF16, isOutput=False)
    t2w = nc.declare_dram_parameter("t2w", [32, 16], BF16, isOutput=False)
    t2b = nc.declare_dram_parameter("t2b", [16, 1], F32, isOutput=False)
    t3w = nc.declare_dram_parameter("t3w", [16, 8], BF16, isOutput=False)
    t3b = nc.declare_dram_parameter("t3b", [8, 1], F32, isOutput=False)
    t4w = nc.declare_dram_parameter("t4w", [8, DZ], BF16, isOutput=False)
    t4bdt = nc.declare_dram_parameter("t4bdt", [DZ, 1], F32, isOutput=False)  # DT * t4_b
    dw1 = nc.declare_dram_parameter("dw1", [DZ, ENC_H], BF16, isOutput=False)
    db1r = nc.declare_dram_parameter("db1r", [1, ENC_H], F32, isOutput=False)
    dw2b = nc.declare_dram_parameter("dw2b", [ENC_H, XD], BF16, isOutput=False)
    db2r = nc.declare_dram_parameter("db2r", [1, XD], BF16, isOutput=False)
    xrec = nc.declare_dram_parameter("x_recon", [BS, T, XD], F32, isOutput=True)

    with tile.TileContext(nc) as tc:
        cpool = tc.alloc_tile_pool(name="const", bufs=1)
        xtpool = tc.alloc_tile_pool(name="xt", bufs=1)
        h1pool = tc.alloc_tile_pool(name="h1", bufs=6)
        h2pool = tc.alloc_tile_pool(name="h2", bufs=6)
        xopool = tc.alloc_tile_pool(name="xo", bufs=6)
        zpool = tc.alloc_tile_pool(name="zwork", bufs=2)
        bpool = tc.alloc_tile_pool(name="bwork", bufs=4)
        psA = tc.alloc_tile_pool(name="psA", bufs=4, space="PSUM")
        psB = tc.alloc_tile_pool(name="psB", bufs=2, space="PSUM")
        psZ = tc.alloc_tile_pool(name="psZ", bufs=1, space="PSUM")

        # ---- load constants into SBUF ----
        const_dmas = []

        def load_xT(g):
            xt_t = xtpool.tile([128, KX, GN], BF16, tag="xtg")
            nc.sync.dma_start(out=xt_t[:],
                              in_=xTb.rearrange("(k p) r -> p k r", p=128)[:, :, GN * g:GN * (g + 1)])
            return xt_t

        w1T_s = cpool.tile([128, KX, MH, 128], BF16)
        const_dmas.append(nc.sync.dma_start(out=w1T_s[:], in_=w1T.rearrange("(k p) (m c) -> p k m c", p=128, c=128)))
        dw2_s = cpool.tile([128, MH, XD], BF16)
        const_dmas.append(nc.sync.dma_start(out=dw2_s[:], in_=dw2b.rearrange("(k p) d -> p k d", p=128)))

        def load_const(ap, shape, dtype=F32):
            t_ = cpool.tile(shape, dtype, tag=f"c_{ap.name}")
            const_dmas.append(nc.sync.dma_start(out=t_[:], in_=ap[:]))
            return t_

        ew2mu_s = cpool.tile([128, MH, DZ], BF16)
        const_dmas.append(nc.sync.dma_start(out=ew2mu_s[:], in_=ew2mu.rearrange("(m p) c -> p m c", p=128)))
        ew2ls_s = cpool.tile([128, MH, DZ], BF16)
        const_dmas.append(nc.sync.dma_start(out=ew2ls_s[:], in_=ew2ls.rearrange("(m p) c -> p m c", p=128)))
        ezT_s = load_const(ezT, [DZ, R])
        ebT_s = load_const(ebT, [DB, R])
        eb2mu_s = load_const(eb2mu, [DZ, 1])
        eb2ls_s = load_const(eb2ls, [DZ, 1])
        whhT_s = load_const(whhT, [HID, 4 * HID], BF16)
        wihT_s = load_const(wihT, [DZ + 1, 4 * HID], BF16)
        fcr2_s = load_const(fcr2, [HID, 35], BF16)
        fcbmu_s = load_const(fcbmu, [DB, 1])
        fcbls_s = load_const(fcbls, [DB, 1])
        twz5_s = load_const(twz5, [DZ + 2, 32], BF16)
        twb_s = load_const(twb, [DB, 32], BF16)
        t2w_s = load_const(t2w, [32, 16], BF16)
        t2b_s = load_const(t2b, [16, 1])
        t3w_s = load_const(t3w, [16, 8], BF16)
        t3b_s = load_const(t3b, [8, 1])
        t4w_s = load_const(t4w, [8, DZ], BF16)
        t4bdt_s = load_const(t4bdt, [DZ, 1])
        dw1_s = load_const(dw1, [DZ, ENC_H], BF16)
        db1r_s = load_const(db1r, [1, ENC_H])
        db2r_s = load_const(db2r, [1, XD], BF16)

        # eb1 / db1 as per-partition columns: [128, MH] where col m = bias block m
        eb1c_s = cpool.tile([128, MH], F32)
        const_dmas.append(nc.sync.dma_start(out=eb1c_s[:], in_=eb1r.rearrange("o (m c) -> (o c) m", c=128)))
        db1c_s = cpool.tile([128, MH], F32)
        const_dmas.append(nc.sync.dma_start(out=db1c_s[:], in_=db1r.rearrange("o (m c) -> (o c) m", c=128)))

        # persistent state / small constants
        ones16b = cpool.tile([1, 16], BF16)
        nc.vector.memset(ones16b[:], 1.0)
        # z / z_next split per row-group so group g+1 writes don't WAR-serialize
        # against group g phase-B reads under tile-granular dep tracking
        zT_g, znT_g, gz_g = [], [], []
        for g in range(NG):
            zt = cpool.tile([DZ + 2, GN], BF16, tag=f"zT{g}")
            const_dmas.append(nc.sync.dma_start(out=zt[DZ:DZ + 1, :], in_=onesd[:]))  # ones row
            const_dmas.append(nc.sync.dma_start(out=zt[DZ + 1:DZ + 2, :],
                                                in_=uTf[:, GN * g:GN * (g + 1)]))      # u row
            zT_g.append(zt)
            znt = cpool.tile([DZ, GN], BF16, tag=f"znT{g}")
            znT_g.append(znt)
            gzt = cpool.tile([128, 4 * GN], BF16, tag=f"gz{g}")
            gz_g.append(gzt)
        h_st = cpool.tile([HID, BS], BF16)
        c_st = cpool.tile([HID, BS], F32)
        nc.vector.memset(h_st[:], 0.0)
        nc.vector.memset(c_st[:], 0.0)
        # absorb each const-DMA completion into a single-wait NOP on the sync
        # engine (per-instruction wait limit is 2), then one all-engine barrier
        # making all consts transitively visible to every later instruction
        for d in const_dmas:
            n_ = nc.sync.nop()
            add_dep_helper(n_.ins, d.ins, sync=True, reason="const-dma absorb")
        tc.strict_bb_all_engine_barrier()


        # ---------- phase A for one row-group ----------
        def phase_a(g, xt_t):
            cols = slice(GN * g, GN * (g + 1))
            h1_tiles = []
            for m in range(MH):
                ps = psA.tile([128, GN], F32, tag="big")
                for k in range(KX):
                    nc.tensor.matmul(ps[:], w1T_s[:, k, m, :], xt_t[:, k, :],
                                     start=(k == 0), stop=(k == KX - 1))
                h1 = h1pool.tile([128, GN], BF16, tag="h1")
                nc.scalar.activation(h1[:], ps[:], AF.Relu, bias=eb1c_s[:, m:m + 1])
                h1_tiles.append(h1)
            # z head: separate mu/ls psums (base-0 partitions only)
            psmu = psZ.tile([DZ, GN], F32, tag="zhmu")
            psls = psZ.tile([DZ, GN], F32, tag="zhls")
            for m in range(MH):
                nc.tensor.matmul(psmu[:], ew2mu_s[:, m, :],
                                 h1_tiles[m][:], start=(m == 0), stop=(m == MH - 1))
            for m in range(MH):
                nc.tensor.matmul(psls[:], ew2ls_s[:, m, :],
                                 h1_tiles[m][:], start=(m == 0), stop=(m == MH - 1))
            s_ls = zpool.tile([DZ, GN], F32, tag="zs")
            nc.scalar.activation(s_ls[:], psls[:], AF.Sigmoid, bias=eb2ls_s[:])
            d1 = zpool.tile([DZ, GN], F32, tag="zd")
            nc.vector.tensor_scalar(d1[:], s_ls[:], -1.0, 1.0, ALU.mult, ALU.add)
            nc.vector.reciprocal(d1[:], d1[:])
            nc.vector.tensor_mul(d1[:], s_ls[:], d1[:])          # exp(ls)
            nc.vector.tensor_mul(d1[:], d1[:], ezT_s[:, cols])   # exp(ls)*eps
            nc.vector.scalar_tensor_tensor(zT_g[g][0:DZ, :], d1[:], eb2mu_s[:],
                                           psmu[:], ALU.add, ALU.add)
            # batched W_ih*z + bias for all 32 steps of this group:
            # gz_g layout [128, step*64 + m*16 + b]
            for m in range(4):
                psz2 = psA.tile([128, GN], F32, tag="big")
                nc.tensor.matmul(psz2[:], wihT_s[:, 128 * m:128 * (m + 1)],
                                 zT_g[g][0:DZ + 1, :], start=True, stop=True)
                dst = gz_g[g][:].rearrange("p (s x) -> p s x", x=64)[:, :, 16 * m:16 * (m + 1)]
                nc.vector.tensor_copy(dst, psz2[:].rearrange("p (s x) -> p s x", x=16))

        # ---------- phase B single step ----------
        def phase_b(t):
            with tc.high_priority():
                _phase_b(t)

        def _phase_b(t):
            g = (BS * t) // GN
            cols = slice(BS * t - GN * g, BS * (t + 1) - GN * g)
            cols2full = slice(BS * t, BS * (t + 1))
            zt = zT_g[g]
            tl = t - g * (GN // BS)
            psg = psB.tile([128, 64], F32, tag="small")
            for m in range(4):
                o = psg[:, 16 * m:16 * (m + 1)]
                nc.tensor.matmul(o, whhT_s[:, 128 * m:128 * (m + 1)], h_st[:], start=True, stop=True)
            gsum = bpool.tile([128, 64], F32, tag="gsum")
            nc.vector.tensor_add(gsum[:], psg[:], gz_g[g][:, 64 * tl:64 * (tl + 1)])
            sig = bpool.tile([128, 48], F32, tag="sig")
            nc.scalar.activation(sig[:], gsum[:, 0:48], AF.Sigmoid)
            gt_ = bpool.tile([128, 16], F32, tag="gt")
            nc.scalar.activation(gt_[:], gsum[:, 48:64], AF.Tanh)
            m1 = bpool.tile([128, 16], F32, tag="m1")
            nc.vector.tensor_mul(m1[:], sig[:, 16:32], c_st[:])
            m2 = bpool.tile([128, 16], F32, tag="m2")
            nc.vector.tensor_mul(m2[:], sig[:, 0:16], gt_[:])
            nc.vector.tensor_add(c_st[:], m1[:], m2[:])
            tc_ = bpool.tile([128, 16], F32, tag="tc")
            nc.scalar.activation(tc_[:], c_st[:], AF.Tanh)
            nc.vector.tensor_mul(h_st[:], sig[:, 32:48], tc_[:])
            # beta head: single MM, mu at partitions 0:3, ls at 32:35
            psb = psB.tile([35, 16], F32, tag="small")
            nc.tensor.matmul(psb[:], fcr2_s[:], h_st[:], start=True, stop=True)
            sl = bpool.tile([DB, 16], F32, tag="sl")
            nc.scalar.activation(sl[:], psb[32:35, :], AF.Sigmoid, bias=fcbls_s[:])
            dd = bpool.tile([DB, 16], F32, tag="dd")
            nc.vector.tensor_scalar(dd[:], sl[:], -1.0, 1.0, ALU.mult, ALU.add)
            nc.vector.reciprocal(dd[:], dd[:])
            nc.vector.tensor_mul(dd[:], sl[:], dd[:])
            nc.vector.tensor_mul(dd[:], dd[:], ebT_s[:, cols])
            beta = bpool.tile([DB, 16], BF16, tag="beta")
            nc.vector.scalar_tensor_tensor(beta[:], dd[:], fcbmu_s[:],
                                           psb[0:DB, :], ALU.add, ALU.add)
            # transition MLP: t1 accumulates z(+bias), u, beta terms
            q1 = psB.tile([32, 16], F32, tag="small")
            nc.tensor.matmul(q1[:], twz5_s[:], zt[:, cols], start=True, stop=False)
            nc.tensor.matmul(q1[:], twb_s[:], beta[:], start=False, stop=True)
            q1r = bpool.tile([32, 16], BF16, tag="q1")
            nc.vector.tensor_scalar(q1r[:], q1[:], 0.0, None, ALU.max)
            q2 = psB.tile([16, 16], F32, tag="small")
            nc.tensor.matmul(q2[:], t2w_s[:], q1r[:], start=True, stop=True)
            q2r = bpool.tile([16, 16], BF16, tag="q2")
            nc.vector.tensor_scalar(q2r[:], q2[:], t2b_s[:], 0.0, ALU.add, ALU.max)
            q3 = psB.tile([8, 16], F32, tag="small")
            nc.tensor.matmul(q3[:], t3w_s[:], q2r[:], start=True, stop=True)
            q3r = bpool.tile([8, 16], BF16, tag="q3")
            nc.vector.tensor_scalar(q3r[:], q3[:], t3b_s[:], 0.0, ALU.add, ALU.max)
            q4 = psB.tile([DZ, 16], F32, tag="small")
            nc.tensor.matmul(q4[:], t4w_s[:], q3r[:], start=True, stop=True)
            zp = bpool.tile([DZ, 16], F32, tag="zp")
            nc.vector.tensor_scalar(zp[:], q4[:], DT_STEP, t4bdt_s[:], ALU.mult, ALU.add)
            nc.vector.tensor_add(znT_g[g][:, cols], zp[:], zt[0:DZ, cols])

        # ---------- phase C: c1 for one row-group ----------
        def phase_c1(g):
            h2_tiles = []
            for m in range(MH):
                ps = psA.tile([128, GN], F32, tag="big")
                nc.tensor.matmul(ps[:], dw1_s[:, 128 * m:128 * (m + 1)],
                                 znT_g[g][:], start=True, stop=True)
                h2 = h2pool.tile([128, GN], BF16, tag="h2")
                nc.scalar.activation(h2[:], ps[:], AF.Relu, bias=db1c_s[:, m:m + 1])
                h2_tiles.append(h2)
            return h2_tiles

        # ---------- phase C: c2 for one row-chunk (128 rows) ----------
        ones128b = cpool.tile([1, 128], BF16)
        nc.vector.memset(ones128b[:], 1.0)

        def phase_c2(ch, h2_tiles):
            # rows 128*ch .. 128*(ch+1); h2_tiles of the containing group
            lo = (128 * ch) % GN           # local offset inside group
            t0 = 128 * ch // BS            # first t of this chunk (8 steps per chunk)
            for n in range(NXD):
                ps = psA.tile([128, 512], F32, tag="big")
                if db2_bias:
                    nc.tensor.matmul(ps[:], ones128b[:], db2r_s[:, 512 * n:512 * (n + 1)],
                                     start=True, stop=False)
                for k in range(MH):
                    nc.tensor.matmul(ps[:], h2_tiles[k][:, lo:lo + 128],
                                     dw2_s[:, k, 512 * n:512 * (n + 1)],
                                     start=(k == 0 and not db2_bias), stop=(k == MH - 1))
                xo = xopool.tile([128, 512], F32, tag="xo")
                nc.scalar.activation(xo[:], ps[:], AF.Sigmoid)
                dst = xrec.rearrange("b t d -> t b d")[t0:t0 + 8, :, 512 * n:512 * (n + 1)]
                nc.sync.dma_start(out=dst, in_=xo[:])

        # ---------- t=63 feedback path ----------
        def t63_path():
            with tc.high_priority():
                _t63_path()

        def _t63_path():
            c62 = slice(BS * 62 - GN, BS * 63 - GN)   # local cols in group 1
            # transposed decode of zn[62]: h2T63 [512,16] as [128, 4*16]
            ps = psB.tile([128, 64], F32, tag="small")
            for m in range(MH):
                nc.tensor.matmul(ps[:, 16 * m:16 * (m + 1)],
                                 dw1_s[:, 128 * m:128 * (m + 1)], znT_g[1][:, c62],
                                 start=True, stop=True)
            h2t = bpool.tile([128, 64], BF16, tag="h2t63")
            for m in range(MH):
                nc.scalar.activation(h2t[:, 16 * m:16 * (m + 1)], ps[:, 16 * m:16 * (m + 1)],
                                     AF.Relu, bias=db1c_s[:, m:m + 1])
            # xhatT63 [4096,16] = [128, 32*16] blocks; psum [128,512]
            psx = psA.tile([128, 512], F32, tag="big")
            for xm in range(KX):
                o = psx[:, 16 * xm:16 * (xm + 1)]
                if db2_bias:
                    nc.tensor.matmul(o, db2r_s[:, 128 * xm:128 * (xm + 1)], ones16b[:],
                                     start=True, stop=False)
                for k in range(MH):
                    nc.tensor.matmul(o, dw2_s[:, k, 128 * xm:128 * (xm + 1)],
                                     h2t[:, 16 * k:16 * (k + 1)],
                                     start=(k == 0 and not db2_bias), stop=(k == MH - 1))
            xh63 = bpool.tile([128, 512], BF16, tag="xh63")
            nc.scalar.activation(xh63[:], psx[:], AF.Sigmoid)
            # transposed encode: h1T63 [512,16] as [128, 4*16]
            pse = psB.tile([128, 64], F32, tag="small")
            for em in range(MH):
                o = pse[:, 16 * em:16 * (em + 1)]
                for k in range(KX):
                    nc.tensor.matmul(o, w1T_s[:, k, em, :], xh63[:, 16 * k:16 * (k + 1)],
                                     start=(k == 0), stop=(k == KX - 1))
            h1t = bpool.tile([128, 64], BF16, tag="h1t63")
            for em in range(MH):
                nc.scalar.activation(h1t[:, 16 * em:16 * (em + 1)], pse[:, 16 * em:16 * (em + 1)],
                                     AF.Relu, bias=eb1c_s[:, em:em + 1])
            # z head 63 (split mu/ls)
            pszm = psB.tile([DZ, 16], F32, tag="small")
            pszl = psB.tile([DZ, 16], F32, tag="small")
            for m in range(MH):
                nc.tensor.matmul(pszm[:], ew2mu_s[:, m, :],
                                 h1t[:, 16 * m:16 * (m + 1)], start=(m == 0), stop=(m == MH - 1))
            for m in range(MH):
                nc.tensor.matmul(pszl[:], ew2ls_s[:, m, :],
                                 h1t[:, 16 * m:16 * (m + 1)], start=(m == 0), stop=(m == MH - 1))
            s63 = bpool.tile([DZ, 16], F32, tag="s63")
            nc.scalar.activation(s63[:], pszl[:], AF.Sigmoid, bias=eb2ls_s[:])
            d63 = bpool.tile([DZ, 16], F32, tag="d63")
            nc.vector.tensor_scalar(d63[:], s63[:], -1.0, 1.0, ALU.mult, ALU.add)
            nc.vector.reciprocal(d63[:], d63[:])
            nc.vector.tensor_mul(d63[:], s63[:], d63[:])
            nc.vector.tensor_mul(d63[:], d63[:], ezT_s[:, BS * 63:BS * 64])
            nc.vector.scalar_tensor_tensor(zT_g[1][0:DZ, BS * 63 - GN:BS * 64 - GN], d63[:],
                                           eb2mu_s[:], pszm[:], ALU.add, ALU.add)

        # ---------- schedule ----------
        for _rep in range(repeats):
            xt0 = load_xT(0)
            phase_a(0, xt0)
            xt1 = load_xT(1)
            for t in range(32):
                phase_b(t)
            phase_a(1, xt1)
            h2a = phase_c1(0)
            for ch in range(4):
                phase_c2(ch, h2a)
            for t in range(32, 63):
                phase_b(t)
            t63_path()
            phase_b(63)
            h2b = phase_c1(1)
            for ch in range(4, 8):
                phase_c2(ch, h2b)

        for p in (psZ, psB, psA, bpool, zpool, xopool, h2pool, h1pool, xtpool, cpool):
            p.release()
    return nc


_NC_CACHE = {}


def _get_nc(db2_bias: bool = True):
    key = (1, db2_bias)
    if key not in _NC_CACHE:
        nc = build_nc(db2_bias=db2_bias)
        nc.finalize()   # runs bacc compile passes (wait legalization, reg alloc)
        _NC_CACHE[key] = nc
    return _NC_CACHE[key]


def _prep_core_inputs(x, u, eps_z, eps_beta, weights):
    """Build per-core input maps. x:[128,64,4096] etc."""
    bf = ml_dtypes.bfloat16
    (enc_w1, enc_b1, enc_w2, enc_b2, dec_w1, dec_b1, dec_w2, dec_b2,
     w_ih, w_hh, b_ih, b_hh, fc_w, fc_b,
     t1_w, t1_b, t2_w, t2_b, t3_w, t3_b, t4_w, t4_b) = weights

    # gate reorder i,f,g,o -> i,f,o,g
    def gate_reord(a, axis):
        parts = np.split(np.asarray(a, np.float32), 4, axis=axis)
        return np.concatenate([parts[0], parts[1], parts[3], parts[2]], axis=axis)

    ew2 = np.asarray(enc_w2, np.float32)           # cols (mu0,ls0,mu1,ls1,mu2,ls2)
    eb2 = np.asarray(enc_b2, np.float32)
    fcw = np.asarray(fc_w, np.float32)
    fcb = np.asarray(fc_b, np.float32)

    whhT = _r2(gate_reord(np.asarray(w_hh, np.float32), 0).T)                 # [128, 512]
    wih = gate_reord(np.asarray(w_ih, np.float32), 0)                         # [512, 3]
    bsum = gate_reord(np.asarray(b_ih, np.float32) + np.asarray(b_hh, np.float32), 0)
    wihT = _r2(np.concatenate([wih.T, bsum[None, :]], axis=0))                # [4, 512]

    t1w = np.asarray(t1_w, np.float32)             # [7, 32] rows: z3,u1,b3
    twz5 = _r2(np.concatenate([t1w[0:3], np.asarray(t1_b, np.float32)[None, :],
                               t1w[3:4]], axis=0).astype(bf))     # [wz; t1_b; wu]
    fcr2 = np.zeros((HID, 35), np.float32)
    fcr2[:, 0:3] = fcw[:, 0::2]
    fcr2[:, 32:35] = fcw[:, 1::2]
    fcr2 = _r2(fcr2.astype(bf))

    common = dict(
        w1T=np.asarray(enc_w1, bf),
        eb1r=_r2(np.asarray(enc_b1, np.float32)[None, :]),
        ew2mu=_r2(ew2[:, 0::2].astype(bf)), ew2ls=_r2(ew2[:, 1::2].astype(bf)),
        eb2mu=_r2(eb2[0::2][:, None]), eb2ls=_r2(eb2[1::2][:, None]),
        whhT=whhT.astype(bf), wihT=wihT.astype(bf),
        fcr2=fcr2, fcbmu=_r2(fcb[0::2][:, None]), fcbls=_r2(fcb[1::2][:, None]),
        twz5=twz5, twb=_r2(t1w[4:7].astype(bf)),
        onesd=np.ones((1, GN), bf),
        t2w=_r2(np.asarray(t2_w, bf)),
        t2b=_r2(np.asarray(t2_b, np.float32)[:, None]),
        t3w=_r2(np.asarray(t3_w, bf)),
        t3b=_r2(np.asarray(t3_b, np.float32)[:, None]),
        t4w=_r2(np.asarray(t4_w, bf)),
        t4bdt=_r2(DT_STEP * np.asarray(t4_b, np.float32)[:, None]),
        dw1=_r2(np.asarray(dec_w1, bf)),
        db1r=_r2(np.asarray(dec_b1, np.float32)[None, :]),
        dw2b=np.asarray(dec_w2, bf),
        db2r=np.asarray(dec_b2, bf)[None, :],
    )

    in_maps = []
    x = np.asarray(x, np.float32)
    u = np.asarray(u, np.float32)
    eps_z = np.asarray(eps_z, np.float32)
    eps_beta = np.asarray(eps_beta, np.float32)
    for ci in range(N_CORES):
        bs = slice(BS * ci, BS * (ci + 1))
        xs = x[bs]                                        # [16, 64, 4096]
        xT = xs.transpose(2, 1, 0).reshape(XD, R)         # col r = t*16+b
        uTc = _r2(u[bs, :, 0].T)                          # [64, 16]
        uTc[T - 1, :] = 0.0
        m = dict(common)
        m["xTb"] = _r2(xT.astype(bf))
        m["ezT"] = _r2(eps_z[bs].transpose(2, 1, 0).reshape(DZ, R))
        m["ebT"] = _r2(eps_beta[bs].transpose(2, 1, 0).reshape(DB, R))
        m["uTf"] = _r2(uTc.reshape(1, R).astype(bf))
        in_maps.append(m)
    return in_maps


def kernel(x, u, eps_z, eps_beta,
           enc_w1, enc_b1, enc_w2, enc_b2,
           dec_w1, dec_b1, dec_w2, dec_b2,
           w_ih, w_hh, b_ih, b_hh, fc_w, fc_b,
           t1_w, t1_b, t2_w, t2_b, t3_w, t3_b, t4_w, t4_b, H):
    assert int(H) == H_STEPS
    weights = (enc_w1, enc_b1, enc_w2, enc_b2, dec_w1, dec_b1, dec_w2, dec_b2,
               w_ih, w_hh, b_ih, b_hh, fc_w, fc_b,
               t1_w, t1_b, t2_w, t2_b, t3_w, t3_b, t4_w, t4_b)
    in_maps = _prep_core_inputs(x, u, eps_z, eps_beta, weights)
    nc = _get_nc(db2_bias=bool(np.any(np.asarray(dec_b2))))
    res = run_bass_kernel_spmd(nc, in_maps, list(range(N_CORES))).results
    x_recon = np.concatenate([np.asarray(r["x_recon"], np.float32) for r in res], axis=0)
    x = np.asarray(x, np.float32)
    gt_idx = np.clip(np.arange(H_STEPS) + 1, 0, T - 1)
    x_gt = np.ascontiguousarray(x[:, gt_idx])
    return x_recon, x_gt

